# revision 10
# baseline (speedup 1.0000x reference)
"""Causal self-attention (B=4, T=2048, C=1024, H=16, D=64) on 8 TRN2 NeuronCores.

Sharding: data-parallel over batch (4) x tensor-parallel over heads (2 groups
of 8 heads).  Core c handles batch c//2 and heads (c%2)*8 .. (c%2)*8+8.
Each core computes its QKV projection shard, causal attention for its 8
heads, and a partial output projection (row-parallel); the host sums the two
partials per batch and adds b_proj (+ the folded V-bias term b_v @ w_proj).

Schedule: attention runs chunk-outer (query chunk I of 512) / pair-inner
(4 head pairs), emitted as software-pipelined groups (scores -> exp -> mask ->
AV).  Between groups, independent "fill" matmuls (QKV m-tiles, V blocks,
output-projection tiles) are dripped in by a cycle-debt counter so the PE
in-order queue never starves while ACT evaluates exp -- keeping the HAM clock
gate warm (2.4 GHz).  Diagonal blocks are computed at 128-column granularity
(widths 512/384/256/128) with a single shared [128,128] triangular mask.

Device layouts (per core, all contiguous per partition for fat DMA packets):
  xT   [128, 8, 2048]  x[b].T tiled: [p, ci, t], channel = ci*128+p, bf16
  wqk  [128, 8, 8, 128] [p, m, ci, f]: q|k weight columns, feature = m*128+f
  bqk  [128, 8]        q|k bias, feature m*128+p (per-partition for DVE add)
  wv   [128, 8, 512]   [p, ci, f] V weight columns for this head group
  wp   [128, 4, 1024]  [p, cit, f] w_proj rows (head group), row = cit*128+p
  tri  [128, 128]      tri[p, i] = 1 if i >= p (causal mask for diag blocks)
  outT [1024, 2048]    partial (attn @ wp).T before bias, fp32

All matmuls bf16 with fp32 PSUM accumulation.  Scores are computed transposed
(kv on partitions, queries free) so exp(P) feeds the AV matmul directly as
the moving operand; the denominator comes from a ones column in vext.
"""

import os
import sys

for _p in (
    "/root/.axon_site",
    "/root/.axon_site/_ro/trn_rl_repo",
    "/root/.axon_site/_ro/pypackages",
    "/opt/trn_rl_repo",
):
    if os.path.isdir(_p) and _p not in sys.path:
        sys.path.append(_p)

from collections import deque
from contextlib import ExitStack

import ml_dtypes
import numpy as np

import concourse.tile as tile
from concourse import bacc, mybir
from concourse.bass import ds, ts
from concourse.bass_utils import run_bass_kernel_spmd

F32 = mybir.dt.float32
F32R = mybir.dt.float32r
BF16 = mybir.dt.bfloat16
EXP = mybir.ActivationFunctionType.Exp
MULT = mybir.AluOpType.mult

B, T, C, H, D = 4, 2048, 1024, 16, 64
HPC = 8            # heads per core
CS = HPC * D       # 512 sharded channels
NC_ = C // 128     # 8 channel tiles
TB = T // 128      # 16 key blocks
TCH = T // 512     # 4 query chunks
SCALE = 1.0 / np.sqrt(D)

_CACHE = {}


def _build_program():
    nc = bacc.Bacc("TRN2", target_bir_lowering=False, debug=False)

    xT_d = nc.dram_tensor("xT", [128, NC_, T], BF16, kind="ExternalInput")
    wqk_d = nc.dram_tensor("wqk", [128, 8, NC_, 128], BF16, kind="ExternalInput")
    bqk_d = nc.dram_tensor("bqk", [128, 8], F32, kind="ExternalInput")
    wv_d = nc.dram_tensor("wv", [128, NC_, CS], BF16, kind="ExternalInput")
    wp_d = nc.dram_tensor("wp", [128, CS // 128, C], BF16, kind="ExternalInput")
    tri_d = nc.dram_tensor("tri", [128, 128], BF16, kind="ExternalInput")
    outT_d = nc.dram_tensor("outT", [C, T], F32, kind="ExternalOutput")

    with tile.TileContext(nc) as tc, ExitStack() as ctx, \
            nc.allow_low_precision(reason="bf16 matmuls, f32r epilogue"):
        pool_c = ctx.enter_context(tc.tile_pool(name="const", bufs=1))
        pool_x = ctx.enter_context(tc.tile_pool(name="xT", bufs=1))
        pool_w = ctx.enter_context(tc.tile_pool(name="wqk", bufs=1))
        pool_wv = ctx.enter_context(tc.tile_pool(name="wv", bufs=1))
        pool_wp = ctx.enter_context(tc.tile_pool(name="wp", bufs=1))
        pool_qk = ctx.enter_context(tc.tile_pool(name="qkT", bufs=1))
        pool_ve = ctx.enter_context(tc.tile_pool(name="vext", bufs=1))
        pool_y = ctx.enter_context(tc.tile_pool(name="yT", bufs=1))
        pool_p = ctx.enter_context(tc.tile_pool(name="P", bufs=4))
        pool_sb = ctx.enter_context(tc.tile_pool(name="psb", bufs=4))
        pool_rc = ctx.enter_context(tc.tile_pool(name="rec", bufs=2))
        pool_bc = ctx.enter_context(tc.tile_pool(name="bc", bufs=2))
        pool_yn = ctx.enter_context(tc.tile_pool(name="yn", bufs=2))
        pool_o = ctx.enter_context(tc.tile_pool(name="out", bufs=3))
        ps_big = ctx.enter_context(tc.tile_pool(name="psbig", bufs=2, space="PSUM"))
        ps_sm = ctx.enter_context(tc.tile_pool(name="pssm", bufs=2, space="PSUM"))
        ps_y = ctx.enter_context(tc.tile_pool(name="psy", bufs=2, space="PSUM"))

        tri = pool_c.tile([128, 128], BF16)
        nc.sync.dma_start(tri[:], tri_d.ap()[:])
        bqk = pool_c.tile([128, 8], F32)
        nc.sync.dma_start(bqk[:], bqk_d.ap()[:])

        wqk = pool_w.tile([128, 8, NC_, 128], BF16)
        for m in (0, 4):
            nc.sync.dma_start(wqk[:, m], wqk_d.ap()[:, m])
        wv = pool_wv.tile([128, NC_, CS], BF16)
        nc.sync.dma_start(wv[:], wv_d.ap()[:])
        xT = pool_x.tile([128, NC_, T], BF16)
        for tch in range(TCH):
            for ci in range(NC_):
                nc.sync.dma_start(
                    xT[:, ci, ts(tch, 512)], xT_d.ap()[:, ci, ts(tch, 512)]
                )
        for m in (1, 5, 2, 6, 3, 7):
            nc.sync.dma_start(wqk[:, m], wqk_d.ap()[:, m])
        wp = pool_wp.tile([128, CS // 128, C], BF16)
        nc.sync.dma_start(wp[:], wp_d.ap())

        qkT = pool_qk.tile([128, 8, T], BF16)
        vext = pool_ve.tile([128, TB, HPC * 65], BF16)
        nc.vector.memset(
            vext[:].rearrange("p tb (h s) -> p tb h s", s=65)[:, :, :, 64], 1.0
        )
        yT = pool_y.tile([128, CS // 128, T], BF16)
        outT_ap = outT_d.ap().rearrange("(co p) t -> p co t", p=128)

        # ---------------- fill task emitters (independent PE work) --------
        def emit_qk(m, tch):
            ps = ps_sm.tile([128, 512], F32, tag="sm", name="qkps")
            for ci in range(NC_):
                nc.tensor.matmul(
                    ps[:], wqk[:, m, ci], xT[:, ci, ts(tch, 512)],
                    start=(ci == 0), stop=(ci == NC_ - 1),
                )
            nc.vector.tensor_scalar_add(
                qkT[:, m, ts(tch, 512)], ps[:], bqk[:, m : m + 1]
            )

        def emit_v(tb):
            ps = ps_sm.tile([128, 512], F32, tag="sm", name="vps")
            for ci in range(NC_):
                nc.tensor.matmul(
                    ps[:], xT[:, ci, ts(tb, 128)], wv[:, ci],
                    start=(ci == 0), stop=(ci == NC_ - 1),
                )
            nc.vector.tensor_copy(
                out=vext[:, tb].rearrange("p (h s) -> p h s", s=65)[:, :, 0:64],
                in_=ps[:].rearrange("p (h d) -> p h d", d=64),
            )

        def emit_proj(co, tch):
            ps = ps_sm.tile([128, 512], F32, tag="sm", name="projps")
            for cit in range(CS // 128):
                nc.tensor.matmul(
                    ps[:], wp[:, cit, ts(co, 128)], yT[:, cit, ts(tch, 512)],
                    start=(cit == 0), stop=(cit == CS // 128 - 1),
                )
            ot = pool_o.tile([128, 512], F32, tag="out")
            nc.vector.tensor_copy(out=ot[:], in_=ps[:])
            nc.sync.dma_start(outT_ap[:, co, ts(tch, 512)], ot[:])

        # fill queues: base = qk m-tiles + V blocks (v gated one chunk ahead
        # so late attention chunks keep fill work); proj gated on chunk
        # completion and preferred once available.
        fq_base = deque()
        for m in (1, 5, 2, 6, 3, 7):
            for tch in range(TCH):
                fq_base.append(("qk", (m, tch), 4096))
        for tb in range(4, TB):
            fq_base.append(("v", (tb,), 4096))
        fq_proj = deque()
        for tch in range(TCH):
            for co in range(C // 128):
                fq_proj.append(("proj", (co, tch), 2048))

        chunks_done = [0] * TCH  # pairs completed per query chunk
        state = {"debt": 0, "qk": {(m, t) for m in (0, 4) for t in range(TCH)},
                 "v": 3, "I": 0}

        def emit_fill(task):
            kind, args, pe = task
            if kind == "qk":
                emit_qk(*args)
                state["qk"].add(args)
            elif kind == "v":
                emit_v(*args)
                state["v"] = max(state["v"], args[0])
            else:
                emit_proj(*args)
            state["debt"] = max(state["debt"] - pe, -12288)

        def pop_fill_while_debt():
            while state["debt"] > 0:
                if fq_proj and chunks_done[fq_proj[0][1][1]] == 4:
                    emit_fill(fq_proj.popleft())
                elif fq_base and (
                    fq_base[0][0] != "v"
                    or fq_base[0][1][0] <= 4 * state["I"] + 7
                ):
                    emit_fill(fq_base.popleft())
                else:
                    break

        def force_prereqs(p, I):
            state["I"] = I
            need = {(m, t) for m in (p, 4 + p) for t in range(TCH)}
            while (not need <= state["qk"]) or state["v"] < 4 * I + 3:
                emit_fill(fq_base.popleft())

        # ---------------- attention chunk (head pair p, query chunk I) ----
        def att_chunk(p, I):
            q0 = I * 512
            psy = [
                ps_y.tile([65, 512], F32, tag="psy", name=f"psy{hb}")
                for hb in range(2)
            ]

            def scores_off(g):
                # 2 full off-diagonal key blocks j = 2g, 2g+1
                pss, Ptl = [], []
                for hb in range(2):
                    s = ps_big.tile([128, 1024], F32, tag="big", name=f"pss{hb}")
                    for jj in range(2):
                        j = 2 * g + jj
                        nc.tensor.matmul(
                            s[:, ts(jj, 512)],
                            qkT[hb * 64 : hb * 64 + 64, 4 + p, ts(j, 128)],
                            qkT[hb * 64 : hb * 64 + 64, p, ds(q0, 512)],
                        )
                    pss.append(s)
                for hb in range(2):
                    P = pool_p.tile([128, 1024], BF16, tag="P", name=f"P{hb}")
                    nc.scalar.activation(P[:], pss[hb][:], EXP, scale=float(SCALE))
                    Ptl.append(P)
                return Ptl

            def av_off(g, Ptl):
                for hb in range(2):
                    h = 2 * p + hb
                    for jj in range(2):
                        j = 2 * g + jj
                        nc.tensor.matmul(
                            psy[hb][:],
                            vext[:, j, ds(h * 65, 65)],
                            Ptl[hb][:, ts(jj, 512)],
                            start=(I > 0 and j == 0),
                            stop=False,
                        )

            def scores_diag_a():
                # diagonal block r=0: full 512 queries
                pss, Ptl = [], []
                for hb in range(2):
                    s = ps_sm.tile([128, 512], F32, tag="sm", name=f"dsa{hb}")
                    nc.tensor.matmul(
                        s[:],
                        qkT[hb * 64 : hb * 64 + 64, 4 + p, ts(4 * I, 128)],
                        qkT[hb * 64 : hb * 64 + 64, p, ds(q0, 512)],
                        start=True, stop=True,
                    )
                    pss.append(s)
                for hb in range(2):
                    P = pool_p.tile([128, 1024], BF16, tag="P", name=f"Pa{hb}")
                    nc.scalar.activation(
                        P[:, 0:512], pss[hb][:], EXP, scale=float(SCALE)
                    )
                    nc.vector.tensor_tensor(
                        P[:, 0:128], P[:, 0:128], tri[:], MULT
                    )
                    Ptl.append(P)
                return Ptl

            def av_diag_a(Ptl):
                for hb in range(2):
                    h = 2 * p + hb
                    nc.tensor.matmul(
                        psy[hb][:],
                        vext[:, 4 * I, ds(h * 65, 65)],
                        Ptl[hb][:, 0:512],
                        start=(I == 0), stop=False,
                    )

            def scores_diag_b():
                # diagonal blocks r=1..3, widths 384/256/128.  Offsets are
                # bank-aligned (0 / 512 / 768): one matmul output must not
                # straddle a 512-fp32 PSUM bank boundary.
                offs = (0, 512, 768)
                wids = (384, 256, 128)
                pss, Ptl = [], []
                for hb in range(2):
                    s = ps_big.tile([128, 1024], F32, tag="big", name=f"dsb{hb}")
                    for r in (1, 2, 3):
                        nc.tensor.matmul(
                            s[:, ds(offs[r - 1], wids[r - 1])],
                            qkT[hb * 64 : hb * 64 + 64, 4 + p, ts(4 * I + r, 128)],
                            qkT[hb * 64 : hb * 64 + 64, p,
                                ds(q0 + 128 * r, wids[r - 1])],
                            start=True, stop=True,
                        )
                    pss.append(s)
                for hb in range(2):
                    P = pool_p.tile([128, 1024], BF16, tag="P", name=f"Pb{hb}")
                    nc.scalar.activation(
                        P[:, 0:384], pss[hb][:, 0:384], EXP, scale=float(SCALE)
                    )
                    nc.scalar.activation(
                        P[:, 512:896], pss[hb][:, 512:896], EXP,
                        scale=float(SCALE),
                    )
                    for o in offs:
                        nc.vector.tensor_tensor(
                            P[:, ds(o, 128)], P[:, ds(o, 128)], tri[:], MULT
                        )
                    Ptl.append(P)
                return Ptl

            def av_diag_b(Ptl):
                offs = (0, 512, 768)
                wids = (384, 256, 128)
                for hb in range(2):
                    h = 2 * p + hb
                    for r in (1, 2, 3):
                        nc.tensor.matmul(
                            psy[hb][:, ds(128 * r, wids[r - 1])],
                            vext[:, 4 * I + r, ds(h * 65, 65)],
                            Ptl[hb][:, ds(offs[r - 1], wids[r - 1])],
                            start=False, stop=(r == 3),
                        )

            # software pipeline: scores(g+1) emitted before av(g); fill
            # matmuls dripped in whenever ACT exp work outruns PE work.
            seq = []
            for g in range(2 * I):
                seq.append((lambda g=g: scores_off(g),
                            lambda Ptl, g=g: av_off(g, Ptl), 4096, 5504))
            seq.append((scores_diag_a, av_diag_a, 2048, 3456))
            seq.append((scores_diag_b, av_diag_b, 3072, 4480))

            pend = None  # (av_fn, Ptl)
            for s_fn, a_fn, pe_c, act_c in seq:
                Ptl = s_fn()
                state["debt"] += act_c - pe_c
                pop_fill_while_debt()
                if pend is not None:
                    pend[0](pend[1])
                pend = (a_fn, Ptl)
            pend[0](pend[1])

            # epilogue: normalize by the ones-column denominator, write yT.
            # Kept off the DVE critical queue (which feeds masks to AV): the
            # PSUM->SBUF copy runs on ACT, the reciprocal is the fast DVE
            # approximation (~51 ULP, fine for a softmax denominator), and
            # the normalize multiply runs on the otherwise-idle GpSimd.
            for hb in range(2):
                psb = pool_sb.tile([65, 512], F32, tag="psb", name=f"psb{hb}")
                nc.scalar.copy(out=psb[:], in_=psy[hb][:])
                # spread the 512 denominators over 64 partitions so the DVE
                # reciprocal uses 64 lanes (a [1,512] reciprocal costs 3.3us)
                dsp = pool_rc.tile([64, 8], F32, tag="dsp")
                nc.sync.dma_start(dsp[:], psb[64:65, :])
                rsp = pool_rc.tile([64, 8], F32, tag="rsp")
                nc.vector.reciprocal(rsp[:], dsp[:])
                rec = pool_rc.tile([1, 512], F32, tag="rec")
                nc.sync.dma_start(rec[:], rsp[:])
                bc = pool_bc.tile([64, 512], F32, tag="bc")
                nc.gpsimd.partition_broadcast(bc[:], rec[:])
                if hb == 0:
                    nc.gpsimd.tensor_tensor(
                        yT[0:64, p, ds(q0, 512)], psb[0:64, :], bc[:], MULT
                    )
                else:
                    yn = pool_yn.tile([64, 512], BF16, tag="yn")
                    nc.gpsimd.tensor_tensor(yn[:], psb[0:64, :], bc[:], MULT)
                    nc.sync.dma_start(yT[64:128, p, ds(q0, 512)], yn[:])

        # ---------------- emission schedule ----------------
        for m in (0, 4):
            for tch in range(TCH):
                emit_qk(m, tch)
        for tb in range(4):
            emit_v(tb)

        for I in range(TCH):
            for p in range(4):
                force_prereqs(p, I)
                att_chunk(p, I)
                chunks_done[I] += 1
        while fq_base:
            emit_fill(fq_base.popleft())
        while fq_proj:
            emit_fill(fq_proj.popleft())

    nc.compile()
    return nc


def kernel(x, w_qkv, b_qkv, w_proj, b_proj):
    x = np.asarray(x, dtype=np.float32)
    w_qkv = np.asarray(w_qkv, dtype=np.float32)
    b_qkv = np.asarray(b_qkv, dtype=np.float32)
    w_proj = np.asarray(w_proj, dtype=np.float32)
    b_proj = np.asarray(b_proj, dtype=np.float32)

    if "nc" not in _CACHE:
        _CACHE["nc"] = _build_program()
    nc = _CACHE["nc"]

    bf = ml_dtypes.bfloat16
    p_ = np.arange(128)[:, None]
    i_ = np.arange(128)[None, :]
    tri = (i_ >= p_).astype(bf)

    in_maps = []
    for c in range(8):
        b, hg = c // 2, c % 2
        sl = slice(hg * CS, (hg + 1) * CS)
        wq = w_qkv[:, sl]
        wk = w_qkv[:, C + hg * CS : C + (hg + 1) * CS]
        wqk_cat = np.concatenate([wq, wk], axis=1)          # [1024, 1024]
        bqk_cat = np.concatenate(
            [b_qkv[sl], b_qkv[C + hg * CS : C + (hg + 1) * CS]]
        )
        wv = w_qkv[:, 2 * C + hg * CS : 2 * C + (hg + 1) * CS]
        in_maps.append({
            "xT": np.ascontiguousarray(
                x[b].T.reshape(NC_, 128, T).transpose(1, 0, 2)
            ).astype(bf),
            "wqk": np.ascontiguousarray(
                wqk_cat.reshape(NC_, 128, 8, 128).transpose(1, 2, 0, 3)
            ).astype(bf),
            "bqk": np.ascontiguousarray(
                bqk_cat.reshape(8, 128).T
            ).astype(np.float32),
            "wv": np.ascontiguousarray(
                wv.reshape(NC_, 128, CS).transpose(1, 0, 2)
            ).astype(bf),
            "wp": np.ascontiguousarray(
                w_proj[hg * CS : (hg + 1) * CS]
                .reshape(CS // 128, 128, C).transpose(1, 0, 2)
            ).astype(bf),
            "tri": tri,
        })

    _CACHE["in_maps"] = in_maps
    res = run_bass_kernel_spmd(nc, in_maps, core_ids=list(range(8)))

    bias = b_proj + b_qkv[2 * C :] @ w_proj
    out = np.empty((B, T, C), dtype=np.float32)
    for b in range(B):
        out[b] = res.results[2 * b]["outT"].T
        out[b] += res.results[2 * b + 1]["outT"].T
        out[b] += bias
    return out


# revision 11
# speedup vs baseline: 1.7695x; 1.7695x over previous
"""Causal self-attention (B=4, T=2048, C=1024, H=16, D=64) on 8 TRN2 NeuronCores.

Sharding: data-parallel over batch (4) x tensor-parallel over heads (2 groups
of 8 heads).  Core c handles batch c//2 and heads (c%2)*8 .. (c%2)*8+8.
Each core computes its QKV projection shard, causal attention for its 8
heads, and a partial output projection (row-parallel); the host sums the two
partials per batch and adds b_proj (+ the folded V-bias term b_v @ w_proj).

Schedule: attention runs chunk-outer (query chunk I of 512) / pair-inner
(4 head pairs), emitted as software-pipelined groups (scores -> exp -> mask ->
AV).  Between groups, independent "fill" matmuls (QKV m-tiles, V blocks,
output-projection tiles) are dripped in by a cycle-debt counter so the PE
in-order queue never starves while ACT evaluates exp -- keeping the HAM clock
gate warm (2.4 GHz).  Diagonal blocks are computed at 128-column granularity
(widths 512/384/256/128) with a single shared [128,128] triangular mask.

Device layouts (per core, all contiguous per partition for fat DMA packets):
  xT   [128, 8, 2048]  x[b].T tiled: [p, ci, t], channel = ci*128+p, bf16
  wqk  [128, 8, 8, 128] [p, m, ci, f]: q|k weight columns, feature = m*128+f
  bqk  [128, 8]        q|k bias, feature m*128+p (per-partition for DVE add)
  wv   [128, 8, 512]   [p, ci, f] V weight columns for this head group
  wp   [128, 4, 1024]  [p, cit, f] w_proj rows (head group), row = cit*128+p
  tri  [128, 128]      tri[p, i] = 1 if i >= p (causal mask for diag blocks)
  outT [1024, 2048]    partial (attn @ wp).T before bias, fp32

All matmuls bf16 with fp32 PSUM accumulation.  Scores are computed transposed
(kv on partitions, queries free) so exp(P) feeds the AV matmul directly as
the moving operand; the denominator comes from a ones column in vext.
"""

import os
import sys

for _p in (
    "/root/.axon_site",
    "/root/.axon_site/_ro/trn_rl_repo",
    "/root/.axon_site/_ro/pypackages",
    "/opt/trn_rl_repo",
):
    if os.path.isdir(_p) and _p not in sys.path:
        sys.path.append(_p)

from collections import deque
from contextlib import ExitStack

import ml_dtypes
import numpy as np

import concourse.tile as tile
from concourse import bacc, mybir
from concourse.bass import ds, ts
from concourse.bass_utils import run_bass_kernel_spmd

F32 = mybir.dt.float32
F32R = mybir.dt.float32r
BF16 = mybir.dt.bfloat16
EXP = mybir.ActivationFunctionType.Exp
MULT = mybir.AluOpType.mult

B, T, C, H, D = 4, 2048, 1024, 16, 64
HPC = 8            # heads per core
CS = HPC * D       # 512 sharded channels
NC_ = C // 128     # 8 channel tiles
TB = T // 128      # 16 key blocks
TCH = T // 512     # 4 query chunks
SCALE = 1.0 / np.sqrt(D)

_CACHE = {}


def _build_program():
    nc = bacc.Bacc("TRN2", target_bir_lowering=False, debug=False)

    xT_d = nc.dram_tensor("xT", [128, NC_, T], BF16, kind="ExternalInput")
    wqk_d = nc.dram_tensor("wqk", [128, 8, NC_, 128], BF16, kind="ExternalInput")
    bqk_d = nc.dram_tensor("bqk", [128, 8], F32, kind="ExternalInput")
    wv_d = nc.dram_tensor("wv", [128, NC_, CS], BF16, kind="ExternalInput")
    wp_d = nc.dram_tensor("wp", [128, CS // 128, C], BF16, kind="ExternalInput")
    tri_d = nc.dram_tensor("tri", [128, 128], BF16, kind="ExternalInput")
    outT_d = nc.dram_tensor("outT", [C, T], F32, kind="ExternalOutput")

    with tile.TileContext(nc) as tc, ExitStack() as ctx, \
            nc.allow_low_precision(reason="bf16 matmuls, f32r epilogue"):
        pool_c = ctx.enter_context(tc.tile_pool(name="const", bufs=1))
        pool_x = ctx.enter_context(tc.tile_pool(name="xT", bufs=1))
        pool_w = ctx.enter_context(tc.tile_pool(name="wqk", bufs=1))
        pool_wv = ctx.enter_context(tc.tile_pool(name="wv", bufs=1))
        pool_wp = ctx.enter_context(tc.tile_pool(name="wp", bufs=1))
        pool_qk = ctx.enter_context(tc.tile_pool(name="qkT", bufs=1))
        pool_ve = ctx.enter_context(tc.tile_pool(name="vext", bufs=1))
        pool_y = ctx.enter_context(tc.tile_pool(name="yT", bufs=1))
        pool_p = ctx.enter_context(tc.tile_pool(name="P", bufs=4))
        pool_sb = ctx.enter_context(tc.tile_pool(name="psb", bufs=4))
        pool_rc = ctx.enter_context(tc.tile_pool(name="rec", bufs=2))
        pool_bc = ctx.enter_context(tc.tile_pool(name="bc", bufs=2))
        pool_yn = ctx.enter_context(tc.tile_pool(name="yn", bufs=2))
        pool_o = ctx.enter_context(tc.tile_pool(name="out", bufs=3))
        ps_big = ctx.enter_context(tc.tile_pool(name="psbig", bufs=2, space="PSUM"))
        ps_sm = ctx.enter_context(tc.tile_pool(name="pssm", bufs=2, space="PSUM"))
        ps_y = ctx.enter_context(tc.tile_pool(name="psy", bufs=2, space="PSUM"))

        tri = pool_c.tile([128, 128], BF16)
        nc.sync.dma_start(tri[:], tri_d.ap()[:])
        bqk = pool_c.tile([128, 8], F32)
        nc.sync.dma_start(bqk[:], bqk_d.ap()[:])

        wqk = pool_w.tile([128, 8, NC_, 128], BF16)
        for m in (0, 4):
            nc.sync.dma_start(wqk[:, m], wqk_d.ap()[:, m])
        wv = pool_wv.tile([128, NC_, CS], BF16)
        nc.sync.dma_start(wv[:], wv_d.ap()[:])
        xT = pool_x.tile([128, NC_, T], BF16)
        for tch in range(TCH):
            for ci in range(NC_):
                nc.sync.dma_start(
                    xT[:, ci, ts(tch, 512)], xT_d.ap()[:, ci, ts(tch, 512)]
                )
        for m in (1, 5, 2, 6, 3, 7):
            nc.sync.dma_start(wqk[:, m], wqk_d.ap()[:, m])
        wp = pool_wp.tile([128, CS // 128, C], BF16)
        nc.sync.dma_start(wp[:], wp_d.ap())

        qkT = pool_qk.tile([128, 8, T], BF16)
        vext = pool_ve.tile([128, TB, HPC * 65], BF16)
        nc.vector.memset(
            vext[:].rearrange("p tb (h s) -> p tb h s", s=65)[:, :, :, 64], 1.0
        )
        yT = pool_y.tile([128, CS // 128, T], BF16)
        outT_ap = outT_d.ap().rearrange("(co p) t -> p co t", p=128)

        # ---------------- fill task emitters (independent PE work) --------
        def emit_qk(m, tch):
            ps = ps_sm.tile([128, 512], F32, tag="sm", name="qkps")
            for ci in range(NC_):
                nc.tensor.matmul(
                    ps[:], wqk[:, m, ci], xT[:, ci, ts(tch, 512)],
                    start=(ci == 0), stop=(ci == NC_ - 1),
                )
            nc.vector.tensor_scalar_add(
                qkT[:, m, ts(tch, 512)], ps[:], bqk[:, m : m + 1]
            )

        def emit_v(tb):
            ps = ps_sm.tile([128, 512], F32, tag="sm", name="vps")
            for ci in range(NC_):
                nc.tensor.matmul(
                    ps[:], xT[:, ci, ts(tb, 128)], wv[:, ci],
                    start=(ci == 0), stop=(ci == NC_ - 1),
                )
            nc.vector.tensor_copy(
                out=vext[:, tb].rearrange("p (h s) -> p h s", s=65)[:, :, 0:64],
                in_=ps[:].rearrange("p (h d) -> p h d", d=64),
            )

        def emit_proj(co, tch):
            ps = ps_sm.tile([128, 512], F32, tag="sm", name="projps")
            for cit in range(CS // 128):
                nc.tensor.matmul(
                    ps[:], wp[:, cit, ts(co, 128)], yT[:, cit, ts(tch, 512)],
                    start=(cit == 0), stop=(cit == CS // 128 - 1),
                )
            ot = pool_o.tile([128, 512], F32, tag="out")
            nc.vector.tensor_copy(out=ot[:], in_=ps[:])
            nc.sync.dma_start(outT_ap[:, co, ts(tch, 512)], ot[:])

        # fill queues: base = qk m-tiles + V blocks (v gated one chunk ahead
        # so late attention chunks keep fill work); proj gated on chunk
        # completion and preferred once available.
        fq_base = deque()
        for m in (1, 5, 2, 6, 3, 7):
            for tch in range(TCH):
                fq_base.append(("qk", (m, tch), 4096))
        for tb in range(4, TB):
            fq_base.append(("v", (tb,), 4096))
        fq_proj = deque()
        for tch in range(TCH):
            for co in range(C // 128):
                fq_proj.append(("proj", (co, tch), 2048))

        chunks_done = [0] * TCH  # pairs completed per query chunk
        state = {"debt": 0, "qk": {(m, t) for m in (0, 4) for t in range(TCH)},
                 "v": 3, "I": 0}

        def emit_fill(task):
            kind, args, pe = task
            if kind == "qk":
                emit_qk(*args)
                state["qk"].add(args)
            elif kind == "v":
                emit_v(*args)
                state["v"] = max(state["v"], args[0])
            else:
                emit_proj(*args)
            state["debt"] = max(state["debt"] - pe, -12288)

        def pop_fill_while_debt():
            while state["debt"] > 0:
                if fq_proj and chunks_done[fq_proj[0][1][1]] == 4:
                    emit_fill(fq_proj.popleft())
                elif fq_base and (
                    fq_base[0][0] != "v"
                    or fq_base[0][1][0] <= 4 * state["I"] + 7
                ):
                    emit_fill(fq_base.popleft())
                else:
                    break

        def force_prereqs(p, I):
            state["I"] = I
            need = {(m, t) for m in (p, 4 + p) for t in range(TCH)}
            while (not need <= state["qk"]) or state["v"] < 4 * I + 3:
                emit_fill(fq_base.popleft())

        # ---------------- attention chunk (head pair p, query chunk I) ----
        def att_chunk(p, I):
            q0 = I * 512
            psy = [
                ps_y.tile([65, 512], F32, tag="psy", name=f"psy{hb}")
                for hb in range(2)
            ]

            def scores_off(g):
                # 2 full off-diagonal key blocks j = 2g, 2g+1
                pss, Ptl = [], []
                for hb in range(2):
                    s = ps_big.tile([128, 1024], F32, tag="big", name=f"pss{hb}")
                    for jj in range(2):
                        j = 2 * g + jj
                        nc.tensor.matmul(
                            s[:, ts(jj, 512)],
                            qkT[hb * 64 : hb * 64 + 64, 4 + p, ts(j, 128)],
                            qkT[hb * 64 : hb * 64 + 64, p, ds(q0, 512)],
                        )
                    pss.append(s)
                for hb in range(2):
                    P = pool_p.tile([128, 1024], BF16, tag="P", name=f"P{hb}")
                    nc.scalar.activation(P[:], pss[hb][:], EXP, scale=float(SCALE))
                    Ptl.append(P)
                return Ptl

            def av_off(g, Ptl):
                for hb in range(2):
                    h = 2 * p + hb
                    for jj in range(2):
                        j = 2 * g + jj
                        nc.tensor.matmul(
                            psy[hb][:],
                            vext[:, j, ds(h * 65, 65)],
                            Ptl[hb][:, ts(jj, 512)],
                            start=(I > 0 and j == 0),
                            stop=False,
                        )

            def scores_diag_a():
                # diagonal block r=0: full 512 queries
                pss, Ptl = [], []
                for hb in range(2):
                    s = ps_sm.tile([128, 512], F32, tag="sm", name=f"dsa{hb}")
                    nc.tensor.matmul(
                        s[:],
                        qkT[hb * 64 : hb * 64 + 64, 4 + p, ts(4 * I, 128)],
                        qkT[hb * 64 : hb * 64 + 64, p, ds(q0, 512)],
                        start=True, stop=True,
                    )
                    pss.append(s)
                for hb in range(2):
                    P = pool_p.tile([128, 1024], BF16, tag="P", name=f"Pa{hb}")
                    nc.scalar.activation(
                        P[:, 0:512], pss[hb][:], EXP, scale=float(SCALE)
                    )
                    nc.vector.tensor_tensor(
                        P[:, 0:128], P[:, 0:128], tri[:], MULT
                    )
                    Ptl.append(P)
                return Ptl

            def av_diag_a(Ptl):
                for hb in range(2):
                    h = 2 * p + hb
                    nc.tensor.matmul(
                        psy[hb][:],
                        vext[:, 4 * I, ds(h * 65, 65)],
                        Ptl[hb][:, 0:512],
                        start=(I == 0), stop=False,
                    )

            def scores_diag_b():
                # diagonal blocks r=1..3, widths 384/256/128.  Offsets are
                # bank-aligned (0 / 512 / 768): one matmul output must not
                # straddle a 512-fp32 PSUM bank boundary.
                offs = (0, 512, 768)
                wids = (384, 256, 128)
                pss, Ptl = [], []
                for hb in range(2):
                    s = ps_big.tile([128, 1024], F32, tag="big", name=f"dsb{hb}")
                    for r in (1, 2, 3):
                        nc.tensor.matmul(
                            s[:, ds(offs[r - 1], wids[r - 1])],
                            qkT[hb * 64 : hb * 64 + 64, 4 + p, ts(4 * I + r, 128)],
                            qkT[hb * 64 : hb * 64 + 64, p,
                                ds(q0 + 128 * r, wids[r - 1])],
                            start=True, stop=True,
                        )
                    pss.append(s)
                for hb in range(2):
                    P = pool_p.tile([128, 1024], BF16, tag="P", name=f"Pb{hb}")
                    nc.scalar.activation(
                        P[:, 0:384], pss[hb][:, 0:384], EXP, scale=float(SCALE)
                    )
                    nc.scalar.activation(
                        P[:, 512:896], pss[hb][:, 512:896], EXP,
                        scale=float(SCALE),
                    )
                    for o in offs:
                        nc.vector.tensor_tensor(
                            P[:, ds(o, 128)], P[:, ds(o, 128)], tri[:], MULT
                        )
                    Ptl.append(P)
                return Ptl

            def av_diag_b(Ptl):
                offs = (0, 512, 768)
                wids = (384, 256, 128)
                for hb in range(2):
                    h = 2 * p + hb
                    for r in (1, 2, 3):
                        nc.tensor.matmul(
                            psy[hb][:, ds(128 * r, wids[r - 1])],
                            vext[:, 4 * I + r, ds(h * 65, 65)],
                            Ptl[hb][:, ds(offs[r - 1], wids[r - 1])],
                            start=False, stop=(r == 3),
                        )

            # software pipeline: scores(g+1) emitted before av(g); fill
            # matmuls dripped in whenever ACT exp work outruns PE work.
            seq = []
            for g in range(2 * I):
                seq.append((lambda g=g: scores_off(g),
                            lambda Ptl, g=g: av_off(g, Ptl), 4096, 5504))
            seq.append((scores_diag_a, av_diag_a, 2048, 3456))
            seq.append((scores_diag_b, av_diag_b, 3072, 4480))

            pend = None  # (av_fn, Ptl)
            for s_fn, a_fn, pe_c, act_c in seq:
                Ptl = s_fn()
                state["debt"] += act_c - pe_c
                pop_fill_while_debt()
                if pend is not None:
                    pend[0](pend[1])
                pend = (a_fn, Ptl)
            pend[0](pend[1])

            # epilogue: normalize by the ones-column denominator, write yT.
            # Kept off the DVE critical queue (which feeds masks to AV): the
            # PSUM->SBUF copy runs on ACT, the reciprocal is the fast DVE
            # approximation (~51 ULP, fine for a softmax denominator), and
            # the normalize multiply runs on the otherwise-idle GpSimd.
            for hb in range(2):
                psb = pool_sb.tile([65, 512], F32, tag="psb", name=f"psb{hb}")
                nc.scalar.copy(out=psb[:], in_=psy[hb][:])
                # spread the 512 denominators over 64 partitions so the DVE
                # reciprocal uses 64 lanes (a [1,512] reciprocal costs 3.3us)
                dsp = pool_rc.tile([64, 8], F32, tag="dsp")
                nc.sync.dma_start(dsp[:], psb[64:65, :])
                rsp = pool_rc.tile([64, 8], F32, tag="rsp")
                nc.vector.reciprocal(rsp[:], dsp[:])
                rec = pool_rc.tile([1, 512], F32, tag="rec")
                nc.sync.dma_start(rec[:], rsp[:])
                bc = pool_bc.tile([64, 512], F32, tag="bc")
                nc.gpsimd.partition_broadcast(bc[:], rec[:])
                if hb == 0:
                    nc.vector.tensor_tensor(
                        yT[0:64, p, ds(q0, 512)], psb[0:64, :], bc[:], MULT
                    )
                else:
                    yn = pool_yn.tile([64, 512], BF16, tag="yn")
                    nc.vector.tensor_tensor(yn[:], psb[0:64, :], bc[:], MULT)
                    nc.sync.dma_start(yT[64:128, p, ds(q0, 512)], yn[:])

        # ---------------- emission schedule ----------------
        for m in (0, 4):
            for tch in range(TCH):
                emit_qk(m, tch)
        for tb in range(4):
            emit_v(tb)

        for I in range(TCH):
            for p in range(4):
                force_prereqs(p, I)
                att_chunk(p, I)
                chunks_done[I] += 1
        while fq_base:
            emit_fill(fq_base.popleft())
        while fq_proj:
            emit_fill(fq_proj.popleft())

    nc.compile()
    return nc


def kernel(x, w_qkv, b_qkv, w_proj, b_proj):
    x = np.asarray(x, dtype=np.float32)
    w_qkv = np.asarray(w_qkv, dtype=np.float32)
    b_qkv = np.asarray(b_qkv, dtype=np.float32)
    w_proj = np.asarray(w_proj, dtype=np.float32)
    b_proj = np.asarray(b_proj, dtype=np.float32)

    if "nc" not in _CACHE:
        _CACHE["nc"] = _build_program()
    nc = _CACHE["nc"]

    bf = ml_dtypes.bfloat16
    p_ = np.arange(128)[:, None]
    i_ = np.arange(128)[None, :]
    tri = (i_ >= p_).astype(bf)

    in_maps = []
    for c in range(8):
        b, hg = c // 2, c % 2
        sl = slice(hg * CS, (hg + 1) * CS)
        wq = w_qkv[:, sl]
        wk = w_qkv[:, C + hg * CS : C + (hg + 1) * CS]
        wqk_cat = np.concatenate([wq, wk], axis=1)          # [1024, 1024]
        bqk_cat = np.concatenate(
            [b_qkv[sl], b_qkv[C + hg * CS : C + (hg + 1) * CS]]
        )
        wv = w_qkv[:, 2 * C + hg * CS : 2 * C + (hg + 1) * CS]
        in_maps.append({
            "xT": np.ascontiguousarray(
                x[b].T.reshape(NC_, 128, T).transpose(1, 0, 2)
            ).astype(bf),
            "wqk": np.ascontiguousarray(
                wqk_cat.reshape(NC_, 128, 8, 128).transpose(1, 2, 0, 3)
            ).astype(bf),
            "bqk": np.ascontiguousarray(
                bqk_cat.reshape(8, 128).T
            ).astype(np.float32),
            "wv": np.ascontiguousarray(
                wv.reshape(NC_, 128, CS).transpose(1, 0, 2)
            ).astype(bf),
            "wp": np.ascontiguousarray(
                w_proj[hg * CS : (hg + 1) * CS]
                .reshape(CS // 128, 128, C).transpose(1, 0, 2)
            ).astype(bf),
            "tri": tri,
        })

    _CACHE["in_maps"] = in_maps
    res = run_bass_kernel_spmd(nc, in_maps, core_ids=list(range(8)))

    bias = b_proj + b_qkv[2 * C :] @ w_proj
    out = np.empty((B, T, C), dtype=np.float32)
    for b in range(B):
        out[b] = res.results[2 * b]["outT"].T
        out[b] += res.results[2 * b + 1]["outT"].T
        out[b] += bias
    return out
